# revision 22
# baseline (speedup 1.0000x reference)
"""EntityAttentionLayer Trainium2 kernel (v2, software-pipelined).

Data-parallel over batch across 8 NeuronCores (256 batches/core).
Per core, per G-group of 16 batches (1024 entity tokens, 256 query tokens):
  - XT:   entities feature-on-partition via strided DMA  [128, 4dc, 1024] f32r
  - K:    feature-major GEMM  kf [128, 4ec, 1024] bf16
  - Q:    feature-major GEMM straight from strided XT view, qf [128,4,256] bf16
  - V:    token-major GEMM    vt [128, 8tc, 512] bf16
  - BDQ2: block-diagonal Q [128=(2h x 64d), 4c, 8bp, 64=(2s x 2h x 16q)] bf16
  - logits^T: per (bp,c) one bf16 MM: kf[:,c,bp*128:+128].T @ bdq2[:,c,bp,:]
      -> ps_l [128=(2s x 64j), 8bp, 4c, 64x]; mask (incl. -1e30 off-diag
      batch blocks) accumulated via identity-stationary MMs from mneg2 bf16
  - exp (scale 1/8) one ACT -> bde [128, 4c, 8g2, 64] bf16 (full write)
  - sums: per c one-hot-column stationary MMs accumulate into ps_s4 [4, 512]
      -> ACT copy to sbuf -> DMA scatter [128,16] -> max+recip on DVE ->
      DMA gather -> rs_row [1, 2048] -> gpsimd partition_broadcast -> rf
  - attn@V per (g2, c): psum_av [128feat, 512]; extract diag blocks -> ao
  - ao *= rf ; out-proj GEMM f32r -> psum_o [128tok, 512], post-mask, DMA out

Software pipeline: attention tail of group h=g-1 (sums/attnv/rs/oproj) is
interleaved with projections of group g so the PE never idles (keeps the
HAM clock-gate warm).
"""

import os
import sys

import numpy as np

sys.path.insert(0, "/opt/trn_rl_repo")

import concourse.bass as bass
import concourse.mybir as mybir
import concourse.tile as tile
from concourse import bacc, bass_utils

import ml_dtypes

F32 = mybir.dt.float32
F32R = mybir.dt.float32r
BF16 = mybir.dt.bfloat16
AF = mybir.ActivationFunctionType
ALU = mybir.AluOpType
BF16NP = ml_dtypes.bfloat16

BS, NE, NQ = 2048, 64, 16
D = 512
H = 8
HD = 64
NCORES = 8
BPC = BS // NCORES          # 256 batches per core
GB = 16                     # batches per G-group
NG = BPC // GB              # 16 groups
NTOK = BPC * NE             # 16384 entity tokens per core
NQT = BPC * NQ              # 4096 query tokens per core


def build_nc(debug=False):
    nc = bacc.Bacc()
    dbg = {}
    if debug:
        for nm, shp in [
            ("dss", [4, 512]), ("drsT", [128, 16]), ("drow", [1, 2048]),
            ("drf0", [128, 1024]), ("drf1", [128, 1024]),
        ]:
            dbg[nm] = nc.declare_dram_parameter(nm, shp, F32, isOutput=True)
        dbg["dbde"] = nc.declare_dram_parameter("dbde", [128, 2048], BF16, isOutput=True)
        dbg["dkf"] = nc.declare_dram_parameter("dkf", [128, 4096], BF16, isOutput=True)
        dbg["dao"] = nc.declare_dram_parameter("dao", [128, 1024], BF16, isOutput=True)
        dbg["daopre"] = nc.declare_dram_parameter("daopre", [128, 1024], BF16, isOutput=True)
        dbg["dvt"] = nc.declare_dram_parameter("dvt", [128, 4096], BF16, isOutput=True)
        dbg["dbdq"] = nc.declare_dram_parameter("dbdq", [128, 2048], BF16, isOutput=True)
        dbg["dmn"] = nc.declare_dram_parameter("dmn", [128, 2048], BF16, isOutput=True)
        dbg["dpsl"] = nc.declare_dram_parameter("dpsl", [128, 2048], F32, isOutput=True)

    ent = nc.declare_dram_parameter("ent", [D, NTOK], BF16, isOutput=False)
    mneg2 = nc.declare_dram_parameter("mneg2", [128, NG * 8 * 4 * 64], BF16, isOutput=False)
    pmt = nc.declare_dram_parameter("pmt", [NQT], F32, isOutput=False)
    wqT = nc.declare_dram_parameter("wqT", [D, 512], BF16, isOutput=False)
    wkT = nc.declare_dram_parameter("wkT", [D, 512], BF16, isOutput=False)
    wvT = nc.declare_dram_parameter("wvT", [D, 512], BF16, isOutput=False)
    woT = nc.declare_dram_parameter("woT", [512, 512], BF16, isOutput=False)
    ident = nc.declare_dram_parameter("ident", [128, 128], BF16, isOutput=False)
    rsd1 = nc.declare_dram_parameter("rsd1", [NG, 4, 512], F32, isOutput=True)
    rsd2 = nc.declare_dram_parameter("rsd2", [NG, 128, 16], F32, isOutput=True)
    sel4 = nc.declare_dram_parameter("sel4", [128, 16], BF16, isOutput=False)
    out = nc.declare_dram_parameter("out", [NQT, 512], F32, isOutput=True)

    ent_r = ent.rearrange("(dc p) n -> p dc n", p=128)          # [128, 4, 16384]
    mneg2_r = mneg2.rearrange("p (g r) -> p g r", g=NG)         # [128, 16, 2048]
    pmt_r = pmt.rearrange("(o p) -> p o", p=128)                # [128, 32]

    with tile.TileContext(nc) as tc:
        with (
            tc.tile_pool(name="wpool", bufs=1) as wpool,
            tc.tile_pool(name="xtp", bufs=3) as xtp,
            tc.tile_pool(name="kfp", bufs=2) as kfp,
            tc.tile_pool(name="vtp", bufs=2) as vtp,
            tc.tile_pool(name="qfp", bufs=2) as qfp,
            tc.tile_pool(name="mnp", bufs=3) as mnp,
            tc.tile_pool(name="bdep", bufs=2) as bdep,
            tc.tile_pool(name="ssp", bufs=2) as ssp,
            tc.tile_pool(name="rsp", bufs=2) as rsp,
            tc.tile_pool(name="rfp", bufs=2) as rfp,
            tc.tile_pool(name="aop", bufs=2) as aop,
            tc.tile_pool(name="osp", bufs=2) as osp,
            tc.tile_pool(name="persist", bufs=1) as persist,
            tc.tile_pool(name="ps", bufs=2, space="PSUM") as psp,
            tc.tile_pool(name="psl", bufs=1, space="PSUM") as pslp,
            tc.tile_pool(name="avp", bufs=2, space="PSUM") as avp,
        ):
            # ---- constants / weights (loaded once) ----
            wq_t = wpool.tile([128, 4, 512], BF16, tag="wq")
            wk_t = wpool.tile([128, 4, 512], BF16, tag="wk")
            wv_t = wpool.tile([128, 4, 512], BF16, tag="wv")
            wo_t = wpool.tile([128, 4, 512], BF16, tag="wo")
            pm_t = wpool.tile([128, 32], F32, tag="pm")
            id_t = wpool.tile([128, 128], BF16, tag="id")
            sel4_t = wpool.tile([128, 4, 4], BF16, tag="sel4")
            nc.sync.dma_start(id_t, ident[:, :])
            nc.sync.dma_start(sel4_t, sel4.rearrange("p (c m) -> p c m", c=4))
            nc.sync.dma_start(pm_t, pmt_r)
            nc.sync.dma_start(wk_t, wkT.rearrange("(dc p) e -> p dc e", p=128))
            ones_t = wpool.tile([128, 1], BF16, tag="ones")
            nc.vector.memset(ones_t, 1.0)

            # persistent block-diagonal q (off-blocks stay zero forever)
            bdq2s = [
                persist.tile([128, 4, 8, 64], BF16, tag=f"bdq{i}", name=f"bdq{i}")
                for i in range(2)
            ]
            for t in bdq2s:
                nc.vector.memset(t, 0.0)

            # init dummies: absorb init-DMA sem values into PE's clock
            dummy0 = psp.tile([1, 16], F32, tag="ps", name="dummy0")
            for i, t in enumerate([wq_t, wk_t, wv_t, wo_t]):
                corner = t[0:1, 0, 0:1]
                nc.tensor.matmul(
                    dummy0[0:1, i : i + 1], corner, corner, start=True, stop=True
                )
            dummy1 = psp.tile([1, 16], F32, tag="ps", name="dummy1")
            for i, t in enumerate([id_t, sel4_t, ones_t]):
                corner = t[0:1, 0, 0:1] if len(t.shape) == 3 else t[0:1, 0:1]
                nc.tensor.matmul(
                    dummy1[0:1, i : i + 1], corner, corner, start=True, stop=True
                )

            # prologue DMAs for group 0
            xts = {}
            mns = {}
            xts[0] = xtp.tile([128, 4, GB * NE], BF16, tag="xt", name="xt0")
            nc.sync.dma_start(xts[0], ent_r[:, :, 0 : GB * NE])
            mns[0] = mnp.tile([128, 8, 4, 64], BF16, tag="mn", name="mn0")
            nc.sync.dma_start(mns[0], mneg2_r[:, 0, :].rearrange("p (b c x) -> p b c x", b=8, c=4))
            xts[1] = xtp.tile([128, 4, GB * NE], BF16, tag="xt", name="xt1")
            nc.sync.dma_start(xts[1], ent_r[:, :, GB * NE : 2 * GB * NE])
            mns[1] = mnp.tile([128, 8, 4, 64], BF16, tag="mn", name="mn1")
            nc.sync.dma_start(mns[1], mneg2_r[:, 1, :].rearrange("p (b c x) -> p b c x", b=8, c=4))
            nc.sync.dma_start(wq_t, wqT.rearrange("(dc p) e -> p dc e", p=128))
            nc.sync.dma_start(wv_t, wvT.rearrange("(dc p) e -> p dc e", p=128))
            nc.sync.dma_start(wo_t, woT.rearrange("(ec p) o -> p ec o", p=128))

            kfs = {}
            qfs = {}
            vts = {}
            bdes = {}
            rss = {}
            aos = {}
            prev_outs = None

            for it in range(NG + 1):
                g = it if it < NG else -1          # projection group
                h = it - 1                          # attention-tail group

                # ---- DMAs for group g+2 (prefetch depth 2) ----
                if 0 <= g < NG - 2:
                    xts[g + 2] = xtp.tile([128, 4, GB * NE], BF16, tag="xt", name="xtn")
                    nc.sync.dma_start(
                        xts[g + 2], ent_r[:, :, (g + 2) * GB * NE : (g + 3) * GB * NE]
                    )
                    mns[g + 2] = mnp.tile([128, 8, 4, 64], BF16, tag="mn", name="mnn")
                    nc.sync.dma_start(
                        mns[g + 2],
                        mneg2_r[:, g + 2, :].rearrange("p (b c x) -> p b c x", b=8, c=4),
                    )

                # ---- absorber: advance PE's DVE clock past prior tail writes ----
                if prev_outs is not None:
                    dummy = psp.tile([1, 16], F32, tag="ps", name="dummyA")
                    nc.tensor.matmul(
                        dummy[0:1, 0:1],
                        prev_outs[0:1, 0:1],
                        prev_outs[0:1, 0:1],
                        start=True,
                        stop=True,
                    )

                # ---- K projection of group g first: fills the PE window
                # while exp of group g-1 completes on ACT (no dependency) ----
                if g >= 0:
                    xt = xts.pop(g)
                    kf = kfp.tile([128, 4, GB * NE], BF16, tag="kf")
                    kfs[g] = kf
                    for ec in range(4):
                        for fg in range(2):
                            ps_k = psp.tile([128, 512], F32, tag="ps", name="ps_k")
                            for dc in range(4):
                                nc.tensor.matmul(
                                    ps_k,
                                    wk_t[:, dc, ec * 128 : (ec + 1) * 128],
                                    xt[:, dc, fg * 512 : (fg + 1) * 512],
                                    start=(dc == 0),
                                    stop=(dc == 3),
                                )
                            if (ec * 2 + fg) % 2 == 0:
                                nc.vector.tensor_copy(kf[:, ec, fg * 512 : (fg + 1) * 512], ps_k)
                            else:
                                nc.scalar.copy(kf[:, ec, fg * 512 : (fg + 1) * 512], ps_k)

                # ================= attention tail of group h: part 1 =========
                # ---- Q projection of group g (issued before attnv so its
                # qf copies sit early in the DVE FIFO: avoids a psum-WAR
                # stall behind the 16 extract copies) ----
                if g >= 0:
                    xq_v = xt.rearrange("p dc (b t) -> p dc b t", b=GB)
                    qf = qfp.tile([128, 4, GB * NQ], BF16, tag="qf")
                    qfs[g] = qf
                    for ec in range(4):
                        ps_q = psp.tile([128, 256], F32, tag="ps", name="ps_q")
                        for dc in range(4):
                            nc.tensor.matmul(
                                ps_q,
                                wq_t[:, dc, ec * 128 : (ec + 1) * 128],
                                xq_v[:, dc, :, 0:NQ],
                                start=(dc == 0),
                                stop=(dc == 3),
                            )
                        if ec % 2 == 0:
                            nc.vector.tensor_copy(qf[:, ec, :], ps_q)
                        else:
                            nc.scalar.copy(qf[:, ec, :], ps_q)
                    bdq2 = bdq2s[g % 2]
                    qf_v = qf.rearrange("p c (bp s q) -> p c bp s q", s=2, q=NQ)
                    for s in (0, 1):
                        nc.vector.tensor_copy(
                            bdq2[0:64, :, :, s * 32 : s * 32 + 16], qf_v[0:64, :, :, s, :]
                        )
                        nc.vector.tensor_copy(
                            bdq2[64:128, :, :, s * 32 + 16 : s * 32 + 32],
                            qf_v[64:128, :, :, s, :],
                        )

                if h >= 0:
                    bde = bdes.pop(h)
                    # sums: accumulate per-c one-hot-column MMs into [4, 512]
                    ps_s4 = pslp.tile([4, 512], F32, tag="psl", name="ps_s4")
                    for c in range(4):
                        nc.tensor.matmul(
                            ps_s4,
                            sel4_t[:, c, :],
                            bde[:, c, :, :],
                            start=(c == 0),
                            stop=(c == 3),
                            skip_group_check=True,
                        )
                    ss = ssp.tile([4, 512], F32, tag="ss")
                    nc.scalar.copy(ss, ps_s4)
                    nc.sync.dma_start(rsd1[h], ss)
                    rsT = rsp.tile([128, 16], F32, tag="rsT", name="rsT")
                    nc.sync.dma_start(
                        rsT, rsd1.rearrange("g c (r q) -> g (c r) q", q=16)[h]
                    )
                    if debug and h == 0:
                        nc.sync.dma_start(dbg["dss"][:, :], ss)
                        nc.sync.dma_start(dbg["dbde"][:, :], bde.rearrange("p a b c -> p (a b c)"))

                if h >= 0:
                    # attn @ V (+ diag extract)
                    vt = vts.pop(h)
                    ao = aop.tile([128, 4, GB * NQ], BF16, tag="ao")
                    aos[h] = ao
                    for pair in range(4):
                        ps_av = avp.tile([128, 512], F32, tag="av", name="ps_av")
                        for s in range(2):
                            g2 = pair * 2 + s
                            for c in range(4):
                                nc.tensor.matmul(
                                    ps_av[:, s * 256 + c * 64 : s * 256 + (c + 1) * 64],
                                    vt[:, g2, c * 128 : (c + 1) * 128],
                                    bde[:, c, g2, :],
                                    start=True,
                                    stop=True,
                                )
                        for P in (0, 1):
                            for B in (0, 1):
                                src = ps_av.rearrange("p (s c y) -> p s c y", s=2, c=4)[
                                    P * 64 : (P + 1) * 64, :, :,
                                    B * 32 + P * 16 : B * 32 + P * 16 + 16,
                                ]
                                dst = ao[P * 64 : (P + 1) * 64].rearrange(
                                    "p c (g2 y) -> p g2 c y", g2=8
                                )[:, pair * 2 : pair * 2 + 2, :, B * 16 : B * 16 + 16]
                                nc.vector.tensor_copy(dst, src)

                    # reciprocal on [128, 16], then gather back to a row
                    rs_row = rsp.tile([1, 2048], F32, tag="rsrow", name="rs_row")
                    rss[h] = rs_row
                    nc.vector.tensor_scalar(rsT, rsT, 1e-30, None, ALU.max)
                    nc.vector.reciprocal(rsT, rsT)
                    nc.sync.dma_start(rsd2[h], rsT)
                    nc.sync.dma_start(rs_row, rsd2.rearrange("g p q -> g (p q)")[h][None, :])
                    if debug and h == 0:
                        nc.sync.dma_start(dbg["drsT"][:, :], rsT)
                        nc.sync.dma_start(dbg["daopre"][:, :], ao.rearrange("p a b -> p (a b)"))

                # ================= projections of group g ====================
                if g >= 0:
                    # V token-major
                    vt = vtp.tile([128, 8, 512], BF16, tag="vt")
                    vts[g] = vt
                    for tc8 in range(8):
                        ps_v = psp.tile([128, 512], F32, tag="ps", name="ps_v")
                        for dc in range(4):
                            nc.tensor.matmul(
                                ps_v,
                                xt[:, dc, tc8 * 128 : (tc8 + 1) * 128],
                                wv_t[:, dc, :],
                                start=(dc == 0),
                                stop=(dc == 3),
                            )
                        if tc8 % 2 == 0:
                            nc.scalar.copy(vt[:, tc8, :], ps_v)
                        else:
                            nc.vector.tensor_copy(vt[:, tc8, :], ps_v)

                    if debug and g == 0:
                        nc.sync.dma_start(dbg["dkf"][:, :], kf.rearrange("p a b -> p (a b)"))
                        nc.sync.dma_start(dbg["dvt"][:, :], vt.rearrange("p a b -> p (a b)"))
                        nc.sync.dma_start(dbg["dmn"][:, :], mns[0].rearrange("p a b c -> p (a b c)"))

                # ================= logits + exp of group g ===================
                if g >= 0:
                    bdq2 = bdq2s[g % 2]
                    if debug and g == 0:
                        nc.sync.dma_start(dbg["dbdq"][:, :], bdq2.rearrange("p a b c -> p (a b c)"))
                    kf = kfs.pop(g)
                    mn = mns.pop(g)
                    ps_l = pslp.tile([128, 8, 4, 64], F32, tag="psl", name="ps_l")
                    # mask first: each MM covers exactly one full PSUM bank
                    # (start=True marks the whole bank pending-zero, so the
                    # bank-covering mask write must precede the accumulating
                    # logits MMs)
                    for bp0 in range(0, 8, 2):
                        nc.tensor.matmul(
                            ps_l[:, bp0 : bp0 + 2, :, :],
                            id_t,
                            mn[:, bp0 : bp0 + 2, :, :],
                            start=True,
                            stop=False,
                            skip_group_check=True,
                        )
                    for bp in range(8):
                        for c in range(4):
                            nc.tensor.matmul(
                                ps_l[:, bp, c, :],
                                kf[:, c, bp * 128 : (bp + 1) * 128],
                                bdq2[:, c, bp, :],
                                start=False,
                                stop=(bp % 2 == 1 and c == 3),
                                skip_group_check=True,
                            )
                    if debug and g == 0:
                        psl_sb = ssp.tile([128, 8, 4, 64], F32, tag="dbgpsl", name="psl_sb")
                        nc.vector.tensor_copy(psl_sb, ps_l)
                        nc.sync.dma_start(dbg["dpsl"][:, :], psl_sb.rearrange("p a b c -> p (a b c)"))
                    bde = bdep.tile([128, 4, 8, 64], BF16, tag="bde")
                    bdes[g] = bde
                    nc.scalar.activation(
                        bde.rearrange("p c g x -> p g c x"),
                        ps_l,
                        AF.Exp,
                        scale=1.0 / np.sqrt(HD),
                    )

                # ============ attention tail of group h: part 2 ==============
                if h >= 0:
                    # broadcast rs over partitions, scale ao
                    rs_row = rss.pop(h)
                    ao = aos[h]
                    rs_v = rs_row.rearrange("o (c g2 s f q) -> o c g2 s f q", c=4, g2=8, s=2, f=2)
                    rfl = []
                    for F in (0, 1):
                        rf = rfp.tile([128, 4, GB * NQ], F32, tag=f"rsf{F}", name=f"rf{F}")
                        nc.gpsimd.partition_broadcast(
                            rf.rearrange("p c (g2 s q) -> p c g2 s q", s=2, q=16),
                            rs_v[:, :, :, :, F, :],
                            channels=128,
                        )
                        rfl.append(rf)
                    if debug and h == 0:
                        nc.sync.dma_start(dbg["drow"][:, :], rs_row)
                        nc.sync.dma_start(dbg["drf0"][:, :], rfl[0].rearrange("p a b -> p (a b)"))
                        nc.sync.dma_start(dbg["drf1"][:, :], rfl[1].rearrange("p a b -> p (a b)"))
                    nc.vector.tensor_tensor(ao[0:64], ao[0:64], rfl[0][0:64], ALU.mult)
                    nc.vector.tensor_tensor(ao[64:128], ao[64:128], rfl[1][64:128], ALU.mult)
                    if debug and h == 0:
                        nc.sync.dma_start(dbg["dao"][:, :], ao.rearrange("p a b -> p (a b)"))

                # ============ attention tail of group h: part 3 ==============
                if h >= 0:
                    ao = aos.pop(h)
                    q0 = h * GB * NQ
                    for tc2 in range(2):
                        ps_o = avp.tile([128, 512], F32, tag="av", name="ps_o")
                        for ec in range(4):
                            nc.tensor.matmul(
                                ps_o,
                                ao[:, ec, tc2 * 128 : (tc2 + 1) * 128],
                                wo_t[:, ec, :],
                                start=(ec == 0),
                                stop=(ec == 3),
                            )
                        out_s = osp.tile([128, 512], F32, tag="outs", name="out_s")
                        nc.scalar.mul(
                            out_s, ps_o, pm_t[:, h * 2 + tc2 : h * 2 + tc2 + 1]
                        )
                        nc.scalar.dma_start(
                            out[q0 + tc2 * 128 : q0 + (tc2 + 1) * 128, :], out_s
                        )
                        prev_outs = out_s

    nc.finalize()
    return nc


_NC_CACHE = None
RUN_KWARGS = {}
LAST_RESULT = None


def _get_nc():
    global _NC_CACHE
    if _NC_CACHE is None:
        _NC_CACHE = build_nc()
    return _NC_CACHE


def _prep_mneg2(pm_i):
    """pm_i: (BPC, NQ, NE) bool -> [128, NG*8*4*64] bf16 with -1e30 where
    masked or off-diagonal batch-parity block."""
    # (g, bp, s, q, j)
    pmv = pm_i.reshape(NG, 8, 2, NQ, NE)
    mw = np.where(pmv, np.float32(-1e30), np.float32(0.0))
    # target [s, j, g, bp, c, s', hpar, q]
    M = np.full((2, NE, NG, 8, 4, 2, 2, NQ), -1e30, np.float32)
    for s in range(2):
        # diag blocks: (s, j, g, bp, c-broadcast, hpar-broadcast, q)
        M[s, :, :, :, :, s, :, :] = mw[:, :, s, :, :].transpose(3, 0, 1, 2)[
            :, :, :, None, None, :
        ]
    return np.ascontiguousarray(M.reshape(128, NG * 8 * 4 * 64).astype(BF16NP))


def kernel(entities, pre_mask, post_mask, W_in, W_out, b_out):
    entities = np.asarray(entities, dtype=np.float32)
    pre_mask = np.asarray(pre_mask)
    post_mask = np.asarray(post_mask)
    W_in = np.asarray(W_in, dtype=np.float32)
    W_out = np.asarray(W_out, dtype=np.float32)
    b_out = np.asarray(b_out, dtype=np.float32)

    wqT = np.ascontiguousarray(W_in[0:512].T.astype(BF16NP))
    wkT = np.ascontiguousarray(W_in[512:1024].T.astype(BF16NP))
    wvT = np.ascontiguousarray(W_in[1024:1536].T.astype(BF16NP))
    woT = np.ascontiguousarray(W_out.T.astype(BF16NP))
    ident = np.eye(128, dtype=BF16NP)
    sel4 = np.ascontiguousarray(
        np.tile(np.eye(4, dtype=BF16NP).reshape(1, 16), (128, 1))
    )

    in_maps = []
    for i in range(NCORES):
        bsl = slice(i * BPC, (i + 1) * BPC)
        ent_i = np.ascontiguousarray(entities[bsl].reshape(NTOK, D).T.astype(BF16NP))
        pm_i = pre_mask[bsl, :NQ, :]  # (256, 16, 64) True -> masked
        mneg2_i = _prep_mneg2(pm_i)
        pmt_i = np.ascontiguousarray(
            (1.0 - post_mask[bsl].astype(np.float32)).reshape(NQT)
        )
        in_maps.append(
            {
                "ent": ent_i,
                "mneg2": mneg2_i,
                "pmt": pmt_i,
                "wqT": wqT,
                "wkT": wkT,
                "wvT": wvT,
                "woT": woT,
                "ident": ident,
                "sel4": sel4,
            }
        )

    nc = _get_nc()
    res = bass_utils.run_bass_kernel_spmd(
        nc, in_maps, list(range(NCORES)), **RUN_KWARGS
    )
    global LAST_RESULT
    LAST_RESULT = res
    outs = [res.results[i]["out"].reshape(BPC, NQ, 512) for i in range(NCORES)]
    full = np.concatenate(outs, axis=0)
    if b_out.any():
        full = full + b_out[None, None, :]
        full = np.where(post_mask[:, :, None], 0.0, full)
    return full.astype(np.float32)
